# revision 15
# baseline (speedup 1.0000x reference)
"""v4: two staggered half-batch chains + K-stacked [h; x] matmuls.

Per chain (128 batch rows), per step: one K=97 matmul per gate computes
z = [h; x; 1] @ [Wh; Wx; b] in a single instruction (no separate
x-projection), into a 3-gate PSUM bank {i,f,o} (one sigmoid ACT) plus a
shared g bank (tanh). The two chains are independent recurrences whose
instructions interleave, so one chain's engine work fills the other's
dependency stalls. h16 is written by the DVE directly into the next step's
stacked rhs tile (rows 0..63); the y slice + ones row is DMAd into rows
64..96. Outputs ride fp16 transposes (fp32-converted on the PSUM->SBUF
copy)."""

import numpy as np

import concourse.bacc as bacc
import concourse.mybir as mybir
from concourse.bass_utils import run_bass_kernel_spmd
from concourse.masks import make_identity
from concourse.tile import TileContext

F32 = mybir.dt.float32
F16 = mybir.dt.float16

B_TOTAL = 256
T_FULL = 2048
D = 32
H = 64
N_CORES = 8
SEG = T_FULL // N_CORES
WARM = 24
HB = 128  # half-batch per chain
K_ST = H + D + 1  # 97: stacked [h; x; 1]

SIG = mybir.ActivationFunctionType.Sigmoid
TANH = mybir.ActivationFunctionType.Tanh

GI, GF, GG, GO = range(4)


def build_nc(seg=SEG, warm=WARM):
    nsteps = seg + warm
    nc = bacc.Bacc()

    yT = nc.dram_tensor("yT", [D + 1, nsteps * B_TOTAL], F16, kind="ExternalInput")
    wcat = nc.dram_tensor("wcat", [K_ST, 4 * H], F16, kind="ExternalInput")
    out = nc.dram_tensor("out", [B_TOTAL, seg, H], F32, kind="ExternalOutput")

    def gcols(g):
        return slice(g * H, (g + 1) * H)

    with TileContext(nc) as tc:
        with (
            tc.tile_pool(name="const", bufs=1) as cons,
            tc.tile_pool(name="xhpool", bufs=4) as xp,
            tc.tile_pool(name="gates", bufs=3) as gp,
            tc.tile_pool(name="ew", bufs=3) as ep,
            tc.tile_pool(name="cpool", bufs=3) as cp,
            tc.tile_pool(name="opool", bufs=4) as osp,
            tc.tile_pool(name="psum", bufs=2, space="PSUM") as pp,
            tc.tile_pool(name="psumt", bufs=2, space="PSUM") as ptp,
        ):
            wc_t = cons.tile([K_ST, 4 * H], F16)
            nc.sync.dma_start(wc_t, wcat[:, :])
            ident = cons.tile([H, H], F16)
            make_identity(nc, ident)
            c0p = cons.tile([H, HB], F32)
            nc.vector.memset(c0p, 0.0)
            c0q = cons.tile([H, HB], F32)
            nc.vector.memset(c0q, 0.0)

            # xh tiles: rows 0..63 = h16 (DVE), rows 64..96 = [y; 1] (DMA)
            def new_xh(ch, k, zero_h):
                xh = xp.tile([K_ST, HB], F16, tag=f"xh{ch}", name=f"xh{ch}_{k}")
                base = k * B_TOTAL + ch * HB
                nc.sync.dma_start(xh[H:K_ST, :], yT[:, base : base + HB])
                if zero_h:
                    nc.vector.memset(xh[0:H, :], 0.0)
                return xh

            xh_cur = [new_xh(0, 0, True), new_xh(1, 0, True)]
            xh_next = [new_xh(0, 1, False), new_xh(1, 1, False)]
            c_prev = [c0p, c0q]

            for k in range(nsteps):
                # g halves of both chains share one PSUM bank
                psG = pp.tile([H, 2 * HB], F32, tag="psG")
                for ch in range(2):
                    xh = xh_cur[ch]
                    psIFO = pp.tile([H, 3 * HB], F32, tag=f"psIFO{ch}")

                    for n, g in enumerate((GI, GF, GO)):
                        nc.tensor.matmul(
                            psIFO[:, n * HB : (n + 1) * HB],
                            wc_t[:, gcols(g)],
                            xh,
                            start=(n == 0),
                            stop=(n == 2),
                            skip_group_check=True,
                        )
                    nc.tensor.matmul(
                        psG[:, ch * HB : (ch + 1) * HB],
                        wc_t[:, gcols(GG)],
                        xh,
                        start=(ch == 0),
                        stop=(ch == 1),
                        skip_group_check=True,
                    )

                    gIFO = gp.tile([H, 3 * HB], F16, tag=f"gIFO{ch}")
                    nc.scalar.activation(gIFO, psIFO[:, :], SIG)
                    gG = ep.tile([H, HB], F16, tag=f"gG{ch}")
                    nc.scalar.activation(gG, psG[:, ch * HB : (ch + 1) * HB], TANH)

                    cf = ep.tile([H, HB], F32, tag=f"cf{ch}")
                    nc.vector.tensor_mul(cf, gIFO[:, HB : 2 * HB], c_prev[ch])
                    m = ep.tile([H, HB], F16, tag=f"m{ch}")
                    nc.vector.tensor_mul(m, gIFO[:, 0:HB], gG)
                    c_new = cp.tile([H, HB], F32, tag=f"c{ch}")
                    nc.vector.tensor_add(c_new, cf, m)
                    tau = ep.tile([H, HB], F16, tag=f"tau{ch}")
                    nc.scalar.activation(tau, c_new, TANH)
                    # h16 straight into the next step's stacked rhs
                    if xh_next[ch] is not None:
                        h_dst = xh_next[ch][0:H, :]
                    else:  # last step: nothing consumes h, but output does
                        h_last = ep.tile([H, HB], F16, tag=f"hl{ch}")
                        h_dst = h_last
                    nc.vector.tensor_mul(h_dst, gIFO[:, 2 * HB : 3 * HB], tau)

                    if k >= warm:
                        t_out = k - warm
                        tp_t = ptp.tile([HB, H], F16, tag="tp")
                        nc.tensor.transpose(tp_t, h_dst, ident)
                        ost = osp.tile([HB, H], F32, tag="ost")
                        nc.vector.tensor_copy(ost, tp_t)
                        nc.sync.dma_start(
                            out[ch * HB : (ch + 1) * HB, t_out, :], ost
                        )

                    c_prev[ch] = c_new

                xh_cur = xh_next
                if k + 2 < nsteps:
                    xh_next = [new_xh(0, k + 2, False), new_xh(1, k + 2, False)]
                else:
                    xh_next = [None, None]

    nc.finalize()
    return nc


def _prep_inputs(y, Wx, Wh, b, seg=SEG, warm=WARM):
    y = np.ascontiguousarray(y, dtype=np.float32)
    Wx = np.ascontiguousarray(Wx, dtype=np.float32)
    Wh = np.ascontiguousarray(Wh, dtype=np.float32)
    b = np.ascontiguousarray(b, dtype=np.float32).reshape(1, 4 * H)
    T = y.shape[1]
    nb = y.shape[0]
    nsteps = seg + warm
    wcat = np.concatenate([Wh, Wx, b], axis=0).astype(np.float16)
    yT_full = np.empty((D + 1, T, nb), np.float16)
    yT_full[:D] = y.transpose(2, 1, 0).astype(np.float16)
    yT_full[D] = 1.0
    in_maps = []
    for c in range(N_CORES):
        t0 = c * seg - warm
        yTc = np.zeros((D + 1, nsteps, nb), np.float16)
        lo = max(t0, 0)
        yTc[:, lo - t0 : nsteps] = yT_full[:, lo : t0 + nsteps]
        in_maps.append(
            {"yT": np.ascontiguousarray(yTc.reshape(D + 1, nsteps * nb)), "wcat": wcat}
        )
    return in_maps


_NC_CACHE = {}


def kernel(y, Wx, Wh, b):
    T = y.shape[1]
    seg = T // N_CORES
    key = (seg, WARM)
    if key not in _NC_CACHE:
        _NC_CACHE[key] = build_nc(seg, WARM)
    nc = _NC_CACHE[key]
    in_maps = _prep_inputs(y, Wx, Wh, b, seg, WARM)
    res = run_bass_kernel_spmd(nc, in_maps, core_ids=list(range(N_CORES)))
    return np.concatenate([res.results[c]["out"] for c in range(N_CORES)], axis=1)


# revision 16
# speedup vs baseline: 1.0855x; 1.0855x over previous
"""v4: two staggered half-batch chains + K-stacked [h; x] matmuls.

Per chain (128 batch rows), per step: one K=97 matmul per gate computes
z = [h; x; 1] @ [Wh; Wx; b] in a single instruction (no separate
x-projection), into a 3-gate PSUM bank {i,f,o} (one sigmoid ACT) plus a
shared g bank (tanh). The two chains are independent recurrences whose
instructions interleave, so one chain's engine work fills the other's
dependency stalls. h16 is written by the DVE directly into the next step's
stacked rhs tile (rows 0..63); the y slice + ones row is DMAd into rows
64..96. Outputs ride fp16 transposes (fp32-converted on the PSUM->SBUF
copy)."""

import numpy as np

import concourse.bacc as bacc
import concourse.mybir as mybir
from concourse.bass_utils import run_bass_kernel_spmd
from concourse.masks import make_identity
from concourse.tile import TileContext

F32 = mybir.dt.float32
F16 = mybir.dt.float16

B_TOTAL = 256
T_FULL = 2048
D = 32
H = 64
N_CORES = 8
SEG = T_FULL // N_CORES
WARM = 24
HB = 128  # half-batch per chain
K_ST = H + D + 1  # 97: stacked [h; x; 1]

SIG = mybir.ActivationFunctionType.Sigmoid
TANH = mybir.ActivationFunctionType.Tanh

GI, GF, GG, GO = range(4)


def build_nc(seg=SEG, warm=WARM):
    nsteps = seg + warm
    nc = bacc.Bacc()

    yT = nc.dram_tensor("yT", [D + 1, nsteps * B_TOTAL], F16, kind="ExternalInput")
    wcat = nc.dram_tensor("wcat", [K_ST, 4 * H], F16, kind="ExternalInput")
    out = nc.dram_tensor("out", [B_TOTAL, seg, H], F32, kind="ExternalOutput")

    def gcols(g):
        return slice(g * H, (g + 1) * H)

    with TileContext(nc) as tc:
        with (
            tc.tile_pool(name="const", bufs=1) as cons,
            tc.tile_pool(name="xhpool", bufs=4) as xp,
            tc.tile_pool(name="gates", bufs=3) as gp,
            tc.tile_pool(name="ew", bufs=3) as ep,
            tc.tile_pool(name="cpool", bufs=3) as cp,
            tc.tile_pool(name="opool", bufs=4) as osp,
            tc.tile_pool(name="psum", bufs=2, space="PSUM") as pp,
            tc.tile_pool(name="psumt", bufs=2, space="PSUM") as ptp,
        ):
            wc_t = cons.tile([K_ST, 4 * H], F16)
            nc.sync.dma_start(wc_t, wcat[:, :])
            ident = cons.tile([H, H], F16)
            make_identity(nc, ident)
            c0p = cons.tile([H, HB], F32)
            nc.vector.memset(c0p, 0.0)
            c0q = cons.tile([H, HB], F32)
            nc.vector.memset(c0q, 0.0)

            # xh tiles: rows 0..63 = h16 (DVE), rows 64..96 = [y; 1] (DMA)
            def new_xh(ch, k, zero_h):
                xh = xp.tile([K_ST, HB], F16, tag=f"xh{ch}", name=f"xh{ch}_{k}")
                base = k * B_TOTAL + ch * HB
                # SWDGE queue: keeps the HWDGE (sync) queue free for the
                # output stream — both were contending at ~88% busy
                nc.gpsimd.dma_start(xh[H:K_ST, :], yT[:, base : base + HB])
                if zero_h:
                    nc.vector.memset(xh[0:H, :], 0.0)
                return xh

            xh_cur = [new_xh(0, 0, True), new_xh(1, 0, True)]
            xh_next = [new_xh(0, 1, False), new_xh(1, 1, False)]
            c_prev = [c0p, c0q]

            for k in range(nsteps):
                # g halves of both chains share one PSUM bank
                psG = pp.tile([H, 2 * HB], F32, tag="psG")
                for ch in range(2):
                    xh = xh_cur[ch]
                    psIFO = pp.tile([H, 3 * HB], F32, tag=f"psIFO{ch}")

                    for n, g in enumerate((GI, GF, GO)):
                        nc.tensor.matmul(
                            psIFO[:, n * HB : (n + 1) * HB],
                            wc_t[:, gcols(g)],
                            xh,
                            start=(n == 0),
                            stop=(n == 2),
                            skip_group_check=True,
                        )
                    nc.tensor.matmul(
                        psG[:, ch * HB : (ch + 1) * HB],
                        wc_t[:, gcols(GG)],
                        xh,
                        start=(ch == 0),
                        stop=(ch == 1),
                        skip_group_check=True,
                    )

                    gIFO = gp.tile([H, 3 * HB], F16, tag=f"gIFO{ch}")
                    nc.scalar.activation(gIFO, psIFO[:, :], SIG)
                    gG = ep.tile([H, HB], F16, tag=f"gG{ch}")
                    nc.scalar.activation(gG, psG[:, ch * HB : (ch + 1) * HB], TANH)

                    cf = ep.tile([H, HB], F32, tag=f"cf{ch}")
                    nc.vector.tensor_mul(cf, gIFO[:, HB : 2 * HB], c_prev[ch])
                    m = ep.tile([H, HB], F16, tag=f"m{ch}")
                    nc.vector.tensor_mul(m, gIFO[:, 0:HB], gG)
                    c_new = cp.tile([H, HB], F32, tag=f"c{ch}")
                    nc.vector.tensor_add(c_new, cf, m)
                    tau = ep.tile([H, HB], F16, tag=f"tau{ch}")
                    nc.scalar.activation(tau, c_new, TANH)
                    # h16 straight into the next step's stacked rhs
                    if xh_next[ch] is not None:
                        h_dst = xh_next[ch][0:H, :]
                    else:  # last step: nothing consumes h, but output does
                        h_last = ep.tile([H, HB], F16, tag=f"hl{ch}")
                        h_dst = h_last
                    nc.vector.tensor_mul(h_dst, gIFO[:, 2 * HB : 3 * HB], tau)

                    if k >= warm:
                        t_out = k - warm
                        tp_t = ptp.tile([HB, H], F16, tag="tp")
                        nc.tensor.transpose(tp_t, h_dst, ident)
                        ost = osp.tile([HB, H], F32, tag="ost")
                        nc.vector.tensor_copy(ost, tp_t)
                        nc.sync.dma_start(
                            out[ch * HB : (ch + 1) * HB, t_out, :], ost
                        )

                    c_prev[ch] = c_new

                xh_cur = xh_next
                if k + 2 < nsteps:
                    xh_next = [new_xh(0, k + 2, False), new_xh(1, k + 2, False)]
                else:
                    xh_next = [None, None]

    nc.finalize()
    return nc


def _prep_inputs(y, Wx, Wh, b, seg=SEG, warm=WARM):
    y = np.ascontiguousarray(y, dtype=np.float32)
    Wx = np.ascontiguousarray(Wx, dtype=np.float32)
    Wh = np.ascontiguousarray(Wh, dtype=np.float32)
    b = np.ascontiguousarray(b, dtype=np.float32).reshape(1, 4 * H)
    T = y.shape[1]
    nb = y.shape[0]
    nsteps = seg + warm
    wcat = np.concatenate([Wh, Wx, b], axis=0).astype(np.float16)
    yT_full = np.empty((D + 1, T, nb), np.float16)
    yT_full[:D] = y.transpose(2, 1, 0).astype(np.float16)
    yT_full[D] = 1.0
    in_maps = []
    for c in range(N_CORES):
        t0 = c * seg - warm
        yTc = np.zeros((D + 1, nsteps, nb), np.float16)
        lo = max(t0, 0)
        yTc[:, lo - t0 : nsteps] = yT_full[:, lo : t0 + nsteps]
        in_maps.append(
            {"yT": np.ascontiguousarray(yTc.reshape(D + 1, nsteps * nb)), "wcat": wcat}
        )
    return in_maps


_NC_CACHE = {}


def kernel(y, Wx, Wh, b):
    T = y.shape[1]
    seg = T // N_CORES
    key = (seg, WARM)
    if key not in _NC_CACHE:
        _NC_CACHE[key] = build_nc(seg, WARM)
    nc = _NC_CACHE[key]
    in_maps = _prep_inputs(y, Wx, Wh, b, seg, WARM)
    res = run_bass_kernel_spmd(nc, in_maps, core_ids=list(range(N_CORES)))
    return np.concatenate([res.results[c]["out"] for c in range(N_CORES)], axis=1)
